# revision 1
# baseline (speedup 1.0000x reference)
"""Trainium2 Bass kernel for nn_CoarseMatching (dual-softmax coarse matching).

Computes, for x0/x1 of shape [2, 6400, 256]:
    sim   = x0 @ x1^T / (C * temperature)                       [n, l, s]
    conf  = softmax(sim, axis=2) * softmax(sim, axis=1)
    mask  = (conf > 0.2) & border_valid & mutual-argmax(conf)
    scores= where(mask, conf, 0)

Distribution: the l (query) axis is sharded over 8 NeuronCores (800 rows
per core, both batches). Per core, two phases:
  P1: sim = matmul, E = exp(sim); per-row sums (ACT accum) and partial
      per-column sums (ones-matmul on the tensor engine). Column sums
      are AllReduce'd across the 8 cores in-kernel.
  P2: recompute sim with two extra bf16 contraction rows carrying
      -log(colsum) (split into bf16 hi+lo for precision), then a single
      ACT pass conf = exp(sim2 - log colsum - log rowsum) using the
      per-partition bias slot for -log(rowsum). conf is DMA'd out per
      128-row strip. The same ACT op accumulates per-row conf sums,
      returned as a certificate: sum_s conf >= max_s conf, so any row
      with sum <= threshold provably contributes nothing to mask/scores.

mask/scores are exactly zero whenever no conf exceeds the 0.2 threshold
(guaranteed for random inputs by the certificate, checked on host; a
numpy fallback reproduces the exact reference semantics otherwise).
"""

import os
import sys

import numpy as np

# The Bass kernel executes on the axon-tunneled NeuronCores via PJRT; make
# sure the axon platform stays available even if the caller pinned
# JAX_PLATFORMS=cpu (keep cpu first so the caller's default backend is
# unchanged).
if "jax" not in sys.modules:
    _jp = os.environ.get("JAX_PLATFORMS")
    if _jp and "axon" not in _jp.split(","):
        os.environ["JAX_PLATFORMS"] = _jp + ",axon"

# ---------------------------------------------------------------------------
# BIR post-pass: split instructions with >1 sync wait into single-wait chains.
# The TRN2 ISA carries one wait slot per instruction; this walrus build
# refuses multi-wait BIR instructions instead of splitting them. Splitting is
# semantics-preserving (waits gate dispatch on the engine's serial stream).
# ---------------------------------------------------------------------------
import orjson

_counter = [0]


def _split_bir(bir_json: bytes) -> bytes:
    bir = orjson.loads(bir_json)
    changed = False
    for fn in bir.get("functions", []):
        for bb in fn.get("blocks", []):
            insts = bb.get("instructions", [])
            out = []
            for inst in insts:
                si = inst.get("sync_info")
                waits = (si or {}).get("on_wait") or []
                keep = 0 if inst.get("opcode") == "Matmult" else 1
                if len(waits) > keep:
                    changed = True
                    for w in waits[: len(waits) - keep]:
                        _counter[0] += 1
                        out.append({
                            "debug": inst.get("debug", 0),
                            "engine": inst["engine"],
                            "ins": [],
                            "name": f"splitwait-{_counter[0]}-{inst['name']}",
                            "opcode": "EventSemaphore",
                            "outs": [],
                            "sync_info": {"on_update": [], "on_wait": [w]},
                        })
                    si["on_wait"] = waits[len(waits) - keep:]
                out.append(inst)
            bb["instructions"] = out
    if not changed:
        return bir_json
    return orjson.dumps(bir)


_installed = [False]


def _install_bir_fix():
    if _installed[0]:
        return
    _installed[0] = True
    import concourse.bass_utils as bu
    import concourse.bass2jax as b2j

    orig = bu.compile_bir_kernel

    def patched(bir_json, tmpdir, neff_name="file.neff"):
        return orig(_split_bir(bir_json), tmpdir, neff_name=neff_name)

    bu.compile_bir_kernel = patched
    b2j.compile_bir_kernel = patched


# ---------------------------------------------------------------------------
# Problem constants (hardcoded per spec)
# ---------------------------------------------------------------------------
N, L, S, C = 2, 6400, 6400, 256
THRESHOLD = 0.2
BORDER = 2
TEMPERATURE = 0.1
H0 = W0 = H1 = W1 = 80
NCORES = 8
LSH = L // NCORES                      # 800 rows per core
SCALE2 = 2.0 / (C * TEMPERATURE)       # x0 pre-scale so matmul yields 2*sim

GROUPS = [(g * 1024, 1024) for g in range(6)] + [(6144, 256)]  # 7 ACT groups
STRIPS = [(k * 128, 128) for k in range(6)] + [(768, 32)]      # 7 per batch
NSTRIP = len(STRIPS) * N                                       # 14
NGRP = len(GROUPS)                                             # 7


def _halves(gw):
    return [(0, 512), (512, 512)] if gw == 1024 else [(0, gw)]


def build_kernel(mode="f32r", reps=1, bench_internal=False,
                 do_p1=True, do_p2=True, do_dma=True):
    import concourse.bass as bass
    import concourse.mybir as mybir
    import concourse.tile as tile

    F32 = mybir.dt.float32
    F32R = mybir.dt.float32r
    BF16 = mybir.dt.bfloat16
    DT_MM = {"f32": F32, "f32r": F32R}[mode]
    AF = mybir.ActivationFunctionType

    nc = bass.Bass(trn_type="TRN2", target_bir_lowering=False, debug=False,
                   num_devices=NCORES)

    x0t = nc.dram_tensor("x0t", [N, C, LSH], DT_MM, kind="ExternalInput")
    x1t = nc.dram_tensor("x1t", [N, C, S], DT_MM, kind="ExternalInput")
    conf_kind = "Internal" if bench_internal else "ExternalOutput"
    conf_d = nc.dram_tensor("conf", [N, LSH, S], F32, kind=conf_kind)
    rowstat_d = nc.dram_tensor("rowstat", [128, NSTRIP], F32, kind="ExternalOutput")

    with tile.TileContext(nc) as tc:
        with tc.tile_pool(name="persist", bufs=1) as pp, \
             tc.tile_pool(name="epool", bufs=4) as ep, \
             tc.tile_pool(name="confpool", bufs=6) as cp, \
             tc.tile_pool(name="dram", bufs=1, space="DRAM") as dp:

            # ---- persistent tiles -------------------------------------
            x1s = [[pp.tile([128, S], DT_MM, tag=f"x1_{n}_{kb}", name=f"x1_{n}_{kb}")
                    for kb in range(2)] for n in range(N)]
            x0s = [[pp.tile([128, LSH], DT_MM, tag=f"x0_{n}_{kb}", name=f"x0_{n}_{kb}")
                    for kb in range(2)] for n in range(N)]
            ones_col = pp.tile([128, 128], BF16, tag="ones_col")
            neg_ones = pp.tile([2, 128], BF16, tag="neg_ones")
            # column stats live at partitions {0,32,64,96} x 4 free groups:
            # cidx = n*NGRP+g -> partition 32*(cidx%4), free group cidx//4
            colsum4 = pp.tile([128, 4 * 1024], F32, tag="colsum4")
            lc4 = pp.tile([128, 4 * 1024], F32, tag="lc4")
            # bf16 hi/lo of log(colsum) reuse colsum4's bytes once it's dead
            hi4 = colsum4[:].bitcast(BF16)[:, 0:4 * 1024]
            lo4 = colsum4[:].bitcast(BF16)[:, 4 * 1024:8 * 1024]
            aug = pp.tile([2, NSTRIP * 1024], BF16, tag="aug")  # row0=hi row1=lo
            rowsum_parts = pp.tile([128, NSTRIP * NGRP], F32, tag="rsp")
            confsum_parts = pp.tile([128, NSTRIP * NGRP], F32, tag="csp")
            rowsum_tot = pp.tile([128, NSTRIP], F32, tag="rst")
            neg_lr = pp.tile([128, NSTRIP], F32, tag="nlr")
            confsum_tot = pp.tile([128, NSTRIP], F32, tag="cst")

            cc_in = dp.tile([4, 4 * 1024], F32, tag="cc_in")
            cc_out = dp.tile([4, 4 * 1024], F32, tag="cc_out")

            # ---- loads + consts ---------------------------------------
            for n in range(N):
                for kb in range(2):
                    nc.sync.dma_start(x1s[n][kb][:], x1t[n, kb * 128:(kb + 1) * 128, :])
                    nc.sync.dma_start(x0s[n][kb][:], x0t[n, kb * 128:(kb + 1) * 128, :])
            nc.gpsimd.memset(ones_col[:], 1.0)
            nc.gpsimd.memset(neg_ones[:], -1.0)
            nc.vector.memset(confsum_parts[:], 0.0)
            nc.vector.memset(rowsum_parts[:], 0.0)
            nc.vector.memset(colsum4[:], 1.0)

            for _rep in range(reps):
                if not do_p1:
                    nc.vector.memset(aug[:], 0.0)
                    nc.vector.memset(neg_lr[:], -18.0)
                if not do_p2:
                    nc.vector.memset(confsum_tot[:], 0.0)

                # =========================================================
                # Phase 1: stats (rowsum via ACT accum, colsum via PE ones)
                # =========================================================
                if do_p1:
                    with tc.tile_pool(name="ps1", bufs=3, space="PSUM") as ps1, \
                         tc.tile_pool(name="pc1", bufs=1, space="PSUM") as pc1:
                        for n in range(N):
                            for g, (c0, gw) in enumerate(GROUPS):
                                pcol = pc1.tile([128, 1024], F32, tag="pcol")
                                for i, (l0, rows) in enumerate(STRIPS):
                                    sidx = n * len(STRIPS) + i
                                    psim = ps1.tile([128, 1024], F32, tag="psim")
                                    for h0, hw in _halves(gw):
                                        for kb in range(2):
                                            nc.tensor.matmul(
                                                psim[:rows, h0:h0 + hw],
                                                x0s[n][kb][:, l0:l0 + rows],
                                                x1s[n][kb][:, c0 + h0:c0 + h0 + hw],
                                                start=(kb == 0), stop=(kb == 1))
                                    e = ep.tile([128, 1024], BF16, tag="e")
                                    nc.scalar.activation(
                                        e[:rows, :gw], psim[:rows, :gw], AF.Exp,
                                        scale=0.5,
                                        accum_out=rowsum_parts[:rows,
                                                               sidx * NGRP + g:
                                                               sidx * NGRP + g + 1])
                                    for h0, hw in _halves(gw):
                                        nc.tensor.matmul(
                                            pcol[:, h0:h0 + hw],
                                            ones_col[:rows, :],
                                            e[:rows, h0:h0 + hw],
                                            start=(i == 0),
                                            stop=(i == len(STRIPS) - 1))
                                cidx = n * NGRP + g
                                cp_, cg_ = 32 * (cidx % 4), (cidx // 4) * 1024
                                nc.vector.tensor_copy(
                                    colsum4[cp_:cp_ + 1, cg_:cg_ + gw],
                                    pcol[0:1, :gw])

                    # row stats -> -log(rowsum) per strip
                    nc.vector.reduce_sum(
                        rowsum_tot[:],
                        rowsum_parts[:].rearrange("p (s j) -> p s j", j=NGRP),
                        axis=mybir.AxisListType.X)
                    nc.scalar.activation(neg_lr[:], rowsum_tot[:], AF.Ln)
                    nc.vector.tensor_scalar_mul(neg_lr[:], neg_lr[:], -1.0)

                    # column stats: AllReduce over the 8 cores
                    nc.gpsimd.dma_start(cc_in[:], colsum4[0:128:32, :])
                    nc.gpsimd.collective_compute(
                        "AllReduce", mybir.AluOpType.add,
                        ins=[cc_in[:]], outs=[cc_out[:]],
                        replica_groups=[list(range(NCORES))])
                    nc.gpsimd.dma_start(lc4[0:128:32, :], cc_out[:])
                    for p4 in range(4):
                        nc.scalar.activation(lc4[32 * p4:32 * p4 + 1, :],
                                             lc4[32 * p4:32 * p4 + 1, :], AF.Ln)
                    # split log(colsum) into bf16 hi+lo (scratch: colsum4's
                    # bytes, now dead), then scatter into `aug` rows via
                    # strided DMAs: aug offset cidx*1024, cidx = g4*4 + p4,
                    # cell at partition 32*p4, free group g4.
                    nc.vector.tensor_copy(hi4, lc4[:])
                    nc.vector.scalar_tensor_tensor(
                        lo4, lc4[:], 1.0, hi4,
                        op0=mybir.AluOpType.mult, op1=mybir.AluOpType.subtract)
                    for p4 in range(4):
                        ng = 4 if p4 < 2 else 3
                        for row, srct in ((0, hi4), (1, lo4)):
                            dst = aug[row:row + 1,
                                      p4 * 1024:
                                      p4 * 1024 + (ng - 1) * 4096 + 1024]
                            nc.sync.dma_start(
                                dst.rearrange("o (g t) -> o g t", t=1024)[:, ::4, :],
                                srct[32 * p4:32 * p4 + 1, 0:ng * 1024]
                                .rearrange("o (g t) -> o g t", t=1024))

                # =========================================================
                # Phase 2: conf = exp(2*sim - log colsum - log rowsum)
                # =========================================================
                if do_p2:
                    with tc.tile_pool(name="ps2", bufs=4, space="PSUM") as ps2:
                        for n in range(N):
                            for i, (l0, rows) in enumerate(STRIPS):
                                sidx = n * len(STRIPS) + i
                                for g, (c0, gw) in enumerate(GROUPS):
                                    cidx = n * NGRP + g
                                    psim = ps2.tile([128, 1024], F32, tag="psim2")
                                    for h0, hw in _halves(gw):
                                        for kb in range(2):
                                            nc.tensor.matmul(
                                                psim[:rows, h0:h0 + hw],
                                                x0s[n][kb][:, l0:l0 + rows],
                                                x1s[n][kb][:, c0 + h0:c0 + h0 + hw],
                                                start=(kb == 0), stop=False)
                                        nc.tensor.matmul(
                                            psim[:rows, h0:h0 + hw],
                                            neg_ones[:, :rows],
                                            aug[:, cidx * 1024 + h0:
                                                cidx * 1024 + h0 + hw],
                                            start=False, stop=True)
                                    cchunk = cp.tile([128, 1024], F32, tag="cchunk")
                                    nc.scalar.activation(
                                        cchunk[:rows, :gw],
                                        psim[:rows, :gw], AF.Exp,
                                        scale=1.0,
                                        bias=neg_lr[:rows, sidx:sidx + 1],
                                        accum_out=confsum_parts[:rows,
                                                                sidx * NGRP + g:
                                                                sidx * NGRP + g + 1])
                                    if do_dma:
                                        nc.sync.dma_start(
                                            conf_d[n, l0:l0 + rows, c0:c0 + gw],
                                            cchunk[:rows, :gw])

                    nc.vector.reduce_sum(
                        confsum_tot[:],
                        confsum_parts[:].rearrange("p (s j) -> p s j", j=NGRP),
                        axis=mybir.AxisListType.X)
            nc.sync.dma_start(rowstat_d[:, :], confsum_tot[:])

    return nc


_cache = {}


def _get_kernel(mode="f32r", reps=1, bench_internal=False,
                do_p1=True, do_p2=True, do_dma=True):
    key = (mode, reps, bench_internal, do_p1, do_p2, do_dma)
    if key not in _cache:
        _install_bir_fix()
        _cache[key] = build_kernel(mode, reps, bench_internal, do_p1, do_p2,
                                   do_dma)
    return _cache[key]


def _border_valid_np():
    def grid_valid(h, w):
        ih = np.arange(h)
        iw = np.arange(w)
        vh = (ih >= BORDER) & (ih < h - BORDER)
        vw = (iw >= BORDER) & (iw < w - BORDER)
        return (vh[:, None] & vw[None, :]).reshape(-1)
    v0 = grid_valid(H0, W0)
    v1 = grid_valid(H1, W1)
    return v0[:, None] & v1[None, :]


def run_device(x0, x1, mode="f32r", reps=1):
    """Run the SPMD kernel; returns (conf [N,L,S], rowstat [8,128,NSTRIP])."""
    import jax
    from concourse.bass_utils import run_bass_kernel_spmd
    nc = _get_kernel(mode, reps)

    # bass2jax picks jax.devices() (default backend); steer it to the axon
    # NeuronCores without disturbing the caller's default backend.
    axon_devs = jax.devices("axon")

    x0t = np.ascontiguousarray(
        (np.asarray(x0, np.float32) * np.float32(SCALE2)).transpose(0, 2, 1))
    x1t = np.ascontiguousarray(np.asarray(x1, np.float32).transpose(0, 2, 1))

    in_maps = []
    for k in range(NCORES):
        sh = np.ascontiguousarray(x0t[:, :, k * LSH:(k + 1) * LSH])
        in_maps.append({"x0t": sh, "x1t": x1t})

    _orig_devices = jax.devices
    jax.devices = lambda *a, **k: (list(axon_devs) if not a else _orig_devices(*a, **k))
    try:
        res = run_bass_kernel_spmd(nc, in_maps, core_ids=list(range(NCORES)))
    finally:
        jax.devices = _orig_devices
    conf = np.concatenate([res.results[k]["conf"] for k in range(NCORES)], axis=1)
    rowstat = np.stack([res.results[k]["rowstat"] for k in range(NCORES)])
    return conf, rowstat


def kernel(x0, x1):
    conf, rowstat = run_device(x0, x1)

    mask = np.zeros((N, L, S), dtype=bool)
    scores = np.zeros((N, L, S), dtype=np.float32)

    # Certificate: rowstat >= per-row max of conf. If every row's conf total
    # is safely below THRESHOLD, mask/scores are exactly all-zero.
    if float(np.max(rowstat)) > THRESHOLD * 0.95:
        # Exact reference semantics on our conf (never triggered for randn
        # inputs; kept for full generality).
        valid = _border_valid_np()[None]
        m = (conf > THRESHOLD) & valid
        m &= conf == conf.max(axis=2, keepdims=True)
        m &= conf == conf.max(axis=1, keepdims=True)
        mask = m
        scores = np.where(mask, conf, np.float32(0.0))

    return conf, mask, scores

